# revision 28
# baseline (speedup 1.0000x reference)
"""Trainium2 Bass kernel for nn_ConvexReLUCNN.

Math (identical multilinear form as the reference, reordered):
    reference:  U = unfold(x,3); A = U.G^T (54 GFLOP); out = A.(v-w)
    here:       CS[(q,z),(dh,s)] = sum_{m,p} pd[m,(p,q,z)] * G[m,(dh+2-p,s)]
                     (one 12-matmul PSUM accumulation; i-shift absorbed in
                      shifted G windows, p-sum absorbed in the accumulation)
                W^T[(dh,w), z]   = sum_q CS[(q,z), (dh, w-q)]
                     (12 tiny transpose-matmuls vs stacked identity J;
                      q-shift absorbed in zero-padded CTsum column views)
                out^T[o, b]      = sum_chunks W_chunk^T @ x^T_chunk

Distribution: sharded by image row band. Core i owns output-image rows
h in [8i, 8i+8) (all channels, widths, batches); host sums the 8 partial
outputs (tiny) - no device collectives.

All wire data is bf16, prepared host-side:
  - xt:  x band pre-transposed to [chw=1536, b=512] -> [128, 12*512]
  - vwj: v|w pre-permuted to the padded (p,q,z=c*10+o) layout [128, 2*4*288]
         plus the stacked identity J = [I32;I32;I32] in cols 2304:2336
  - g:   G band rows [8i-2, 8i+8) zero-clipped -> [128, 4*620]
"""

import numpy as np
from contextlib import ExitStack

import ml_dtypes

import concourse.bass as bass
import concourse.mybir as mybir
import concourse.tile as tile
from concourse import bacc
from concourse.bass_utils import run_bass_kernel_spmd

N_CORES = 8
B_FULL = 512
C_CH, H, W = 3, 64, 64
HB = H // N_CORES           # 8 image rows per core
BAND = C_CH * HB * W        # 1536 chw positions per core
M = 512                     # num_neurons
O = 10
Ho = Wo = 62
IW = HB + 2                 # 10 patch-grid rows feeding one band
NL = IW * Wo                # 620 local G columns
Z = 32                      # padded (c,o) block: 3*10 -> 32
KO2 = 9 * Z                 # 288 = 3p x 3q x 32z
NW = HB * Wo                # 496 = shifted-G window (8 rows x 62)
RW = 64                     # CTsum row width: payload s=0..62 at cols 2..64,
                            # so the per-q shifted [32,128] lhsT window is a
                            # flat 1D slice (row wrap hits border zeros)
CTS_LEN = HB * RW + 2       # 514 (+2 tail zeros for the q=0 wrap past row 7)
VW_LEN = 2 * 4 * KO2        # 2304
VWJ_LEN = VW_LEN + Z        # 2336 (+ stacked identity J)
G_LEN = 4 * NL              # 2480
XCHUNKS = 4
TPC = 12 // XCHUNKS         # t12 tiles per x DMA chunk
NWARM = 28                  # PE p-state warmup matmuls (bridge the DMA wait)

F32 = mybir.dt.float32
BF16 = mybir.dt.bfloat16
BF16NP = ml_dtypes.bfloat16

_NC = None


def _build():
    nc = bacc.Bacc("TRN2", target_bir_lowering=False, debug=False,
                   num_devices=N_CORES)
    x_d = nc.dram_tensor("xt", [128, 12 * B_FULL], BF16,
                         kind="ExternalInput").ap()
    v_d = nc.dram_tensor("vwj", [128, VWJ_LEN], BF16,
                         kind="ExternalInput").ap()
    g_d = nc.dram_tensor("g", [128, G_LEN], BF16, kind="ExternalInput").ap()
    o_d = nc.dram_tensor("out", [O, B_FULL], F32, kind="ExternalOutput").ap()

    with tile.TileContext(nc) as tc, ExitStack() as ctx:
        const = ctx.enter_context(tc.tile_pool(name="const", bufs=1))
        big = ctx.enter_context(tc.tile_pool(name="big", bufs=1))
        psW = ctx.enter_context(tc.tile_pool(name="psW", bufs=1, space="PSUM"))
        psC = ctx.enter_context(tc.tile_pool(name="psC", bufs=1, space="PSUM"))
        psT = ctx.enter_context(tc.tile_pool(name="psT", bufs=1, space="PSUM"))
        psF = ctx.enter_context(tc.tile_pool(name="psF", bufs=1, space="PSUM"))

        # ---- PE p-state warmup on junk data (runs during DMA wait) -------
        junk = const.tile([128, 256], BF16)
        nc.vector.memset(junk[:], 0.25)
        pj = psW.tile([128, 192], F32, tag="pj")
        for _ in range(NWARM):
            nc.tensor.matmul(pj[:], junk[:, 0:128], junk[:, 0:192],
                             start=True, stop=True)

        def filler(n):
            for _ in range(n):
                nc.tensor.matmul(pj[:], junk[:, 0:128], junk[:, 0:192],
                                 start=True, stop=True)

        # CTsum per q: rows z, cols (dh, 2+s) with 64-wide rows, zero borders
        CTq = [const.tile([Z, CTS_LEN], BF16, name=f"CTq{q}")
               for q in range(3)]
        for q in range(3):
            nc.vector.memset(CTq[q][:], 0.0)

        # ---- loads: vwj, g (2 halves), x (4 chunks) -- strict FIFO -------
        vwj = big.tile([128, VWJ_LEN], BF16, tag="vwj")
        nc.sync.dma_start(vwj[:], v_d)
        gs = big.tile([128, 4, NL], BF16, tag="gs")
        nc.sync.dma_start(gs[:, 0:2, :], g_d[:, 0:2 * NL])
        nc.sync.dma_start(gs[:, 2:4, :], g_d[:, 2 * NL:4 * NL])
        XT = [big.tile([128, TPC, B_FULL], BF16, tag=f"X{j}", name=f"X{j}")
              for j in range(XCHUNKS)]
        for j in range(XCHUNKS):
            w0 = TPC * B_FULL * j
            nc.sync.dma_start(XT[j][:], x_d[:, w0:w0 + TPC * B_FULL])

        vwv = vwj[:, 0:VW_LEN].rearrange("p (s t k) -> p s t k", s=2, t=4)
        Jt = vwj[:, VW_LEN:VWJ_LEN]             # [96 used, 32] = [I;I;I]

        # ---- pd = v - w in padded (p, q, z) layout, bf16; split per t ----
        pd2 = big.tile([128, 4, KO2], BF16, tag="pd2")
        nc.vector.tensor_sub(pd2[:, 0:2], vwv[:, 0, 0:2], vwv[:, 1, 0:2])
        nc.vector.tensor_sub(pd2[:, 2:4], vwv[:, 0, 2:4], vwv[:, 1, 2:4])

        # ---- CS = sum_{t,p} pd2_tp.T @ G_t[rows 2-p .. 10-p] -------------
        ps = psC.tile([96, NW], F32, tag="psC")
        for t in range(4):
            for p in range(3):
                nc.tensor.matmul(
                    ps[:],
                    pd2[:, t, 96 * p:96 * (p + 1)],
                    gs[:, t, Wo * (2 - p):Wo * (2 - p) + NW],
                    start=(t == 0 and p == 0), stop=(t == 3 and p == 2))
        psv = ps[:].rearrange("p (h s) -> p h s", s=Wo)
        # DVE psum-fp32 -> bf16 casts corrupt data on HW; keep them on ACT.
        cast_eng = [nc.scalar.copy, nc.scalar.copy, nc.scalar.copy]
        for q in range(3):
            CTv = CTq[q][:, 0:HB * RW].rearrange("p (h s) -> p h s", s=RW)
            cast_eng[q](CTv[:, :, 2:2 + Wo], psv[32 * q:32 * (q + 1)])
        filler(2)

        # ---- W^T: 12 tiny transpose-matmuls, q-shift in the lhsT view ----
        # psT[(dh2, w), z] += CTq[q][z, flat (dh2*64 + w - q + 2)] @ I32
        pst = psT.tile([128, 4 * Z], F32, tag="pst")
        # jb-outer: consecutive 3-matmul groups per 32-col region (a start
        # resets the whole bank's accumulate-valid state, so groups must
        # not interleave within the bank).
        WsB = big.tile([128, 4, Z], BF16, tag="WsB")
        for jb in range(4):
            for q in range(3):
                base = 128 * jb + 2 - q
                lhsT = CTq[q][:, base:base + 128]
                nc.tensor.matmul(pst[:, 32 * jb:32 * (jb + 1)],
                                 lhsT, Jt[0:Z, :],
                                 start=(q == 0), stop=(q == 2))
            nc.scalar.copy(WsB[:, jb, :], pst[:, 32 * jb:32 * (jb + 1)])

        # ---- final: out^T[o, b] += W_chunk.T @ x^T_chunk over 12 chunks --
        # Two batch halves: the first half's copy-out + DMA overlap the
        # second half's matmuls.
        BH = B_FULL // 2
        obuf = const.tile([O, B_FULL], F32)
        for h in range(2):
            pf = psF.tile([O, BH], F32, tag="psF", bufs=2)
            for k in range(12):
                jb, c = divmod(k, 3)      # jb-major: first matmuls only
                t12 = 4 * c + jb          # need WsB block jb=0
                nc.tensor.matmul(pf[:],
                                 WsB[:, jb, O * c:O * (c + 1)],
                                 XT[t12 // TPC][:, t12 % TPC,
                                                BH * h:BH * (h + 1)],
                                 start=(k == 0), stop=(k == 11))
            nc.vector.tensor_copy(obuf[:, BH * h:BH * (h + 1)], pf[:])
            nc.sync.dma_start(o_d[:, BH * h:BH * (h + 1)],
                              obuf[:, BH * h:BH * (h + 1)])
    nc.compile()
    return nc


def _get_nc():
    global _NC
    if _NC is None:
        _NC = _build()
    return _NC


def _permute_vw(a):
    """(M, 27, 10) fp32 -> [128, 4, 288] bf16 in (p, q, z=c*10+o) layout."""
    ar = a.reshape(M, 3, 3, 3, O)            # (m, c, p, q, o)
    at = ar.transpose(0, 2, 3, 1, 4).reshape(M, 3, 3, 3 * O)
    ap = np.zeros((M, 3, 3, Z), np.float32)
    ap[..., :3 * O] = at
    return ap.reshape(4, 128, KO2).transpose(1, 0, 2).astype(BF16NP)


def _shard_inputs(inputs):
    x = np.ascontiguousarray(inputs["x"], dtype=np.float32)   # (512,3,64,64)
    G = np.ascontiguousarray(inputs["G"], dtype=np.float32)   # (512,3844)
    vp = _permute_vw(np.asarray(inputs["v"], dtype=np.float32))
    wp = _permute_vw(np.asarray(inputs["w"], dtype=np.float32))
    vw = np.stack([vp, wp], axis=1).reshape(128, VW_LEN)      # [128, 2304]
    Jp = np.zeros((128, Z), np.float32)
    Jp[:96] = np.tile(np.eye(Z, dtype=np.float32), (3, 1))
    vwj = np.ascontiguousarray(
        np.concatenate([vw, Jp.astype(BF16NP)], axis=1))      # [128, 2336]
    Gim = G.reshape(M, Ho, Wo)
    in_maps = []
    for i in range(N_CORES):
        h0 = HB * i
        xb = x[:, :, h0:h0 + HB, :].reshape(B_FULL, BAND)
        xt = np.ascontiguousarray(xb.T).reshape(12, 128, B_FULL)
        xt = np.ascontiguousarray(
            xt.transpose(1, 0, 2)).reshape(128, 12 * B_FULL).astype(BF16NP)
        gsh = np.zeros((M, IW, Wo), np.float32)
        lo, hi = h0 - 2, h0 + HB          # patch-grid rows needed
        clo, chi = max(lo, 0), min(hi, Ho)
        gsh[:, clo - lo:chi - lo, :] = Gim[:, clo:chi, :]
        gb = np.ascontiguousarray(
            gsh.reshape(4, 128, NL).transpose(1, 0, 2).reshape(
                128, G_LEN)).astype(BF16NP)
        in_maps.append({"xt": xt, "vwj": vwj, "g": gb})
    return in_maps


def _run(inputs, trace=False, **kw):
    nc = _get_nc()
    in_maps = _shard_inputs(inputs)
    res = run_bass_kernel_spmd(nc, in_maps, list(range(N_CORES)),
                               trace=trace, **kw)
    acc = np.zeros((O, B_FULL), np.float64)
    for i in range(N_CORES):
        acc += res.results[i]["out"].astype(np.float64)
    return np.ascontiguousarray(acc.T).astype(np.float32), res


def kernel(**inputs) -> np.ndarray:
    return _run(inputs)[0]


# revision 30
# speedup vs baseline: 1.0238x; 1.0238x over previous
"""Trainium2 Bass kernel for nn_ConvexReLUCNN.

Math (identical multilinear form as the reference, reordered):
    reference:  U = unfold(x,3); A = U.G^T (54 GFLOP); out = A.(v-w)
    here:       CS[(q,z),(dh,s)] = sum_{m,p} pd[m,(p,q,z)] * G[m,(dh+2-p,s)]
                     (one 12-matmul PSUM accumulation; i-shift absorbed in
                      shifted G windows, p-sum absorbed in the accumulation)
                W^T[(dh,w), z]   = sum_q CS[(q,z), (dh, w-q)]
                     (12 tiny transpose-matmuls vs stacked identity J;
                      q-shift absorbed in zero-padded CTsum column views)
                out^T[o, b]      = sum_chunks W_chunk^T @ x^T_chunk

Distribution: sharded by image row band. Core i owns output-image rows
h in [8i, 8i+8) (all channels, widths, batches); host sums the 8 partial
outputs (tiny) - no device collectives.

All wire data is bf16, prepared host-side:
  - xt:  x band pre-transposed to [chw=1536, b=512] -> [128, 12*512]
  - vwj: v|w pre-permuted to the padded (p,q,z=c*10+o) layout [128, 2*4*288]
         plus the stacked identity J = [I32;I32;I32] in cols 2304:2336
  - g:   G band rows [8i-2, 8i+8) zero-clipped -> [128, 4*620]
"""

import numpy as np
from contextlib import ExitStack

import ml_dtypes

import concourse.bass as bass
import concourse.mybir as mybir
import concourse.tile as tile
from concourse import bacc
from concourse.bass_utils import run_bass_kernel_spmd

N_CORES = 8
B_FULL = 512
C_CH, H, W = 3, 64, 64
HB = H // N_CORES           # 8 image rows per core
BAND = C_CH * HB * W        # 1536 chw positions per core
M = 512                     # num_neurons
O = 10
Ho = Wo = 62
IW = HB + 2                 # 10 patch-grid rows feeding one band
NL = IW * Wo                # 620 local G columns
Z = 32                      # padded (c,o) block: 3*10 -> 32
KO2 = 9 * Z                 # 288 = 3p x 3q x 32z
NW = HB * Wo                # 496 = shifted-G window (8 rows x 62)
RW = 64                     # CTsum row width: payload s=0..62 at cols 2..64,
                            # so the per-q shifted [32,128] lhsT window is a
                            # flat 1D slice (row wrap hits border zeros)
CTS_LEN = HB * RW + 2       # 514 (+2 tail zeros for the q=0 wrap past row 7)
VW_LEN = 2 * 4 * KO2        # 2304
VWJ_LEN = VW_LEN + Z        # 2336 (+ stacked identity J)
G_LEN = 4 * NL              # 2480
XCHUNKS = 4
TPC = 12 // XCHUNKS         # t12 tiles per x DMA chunk
NWARM = 28                  # PE p-state warmup matmuls (bridge the DMA wait)

F32 = mybir.dt.float32
BF16 = mybir.dt.bfloat16
BF16NP = ml_dtypes.bfloat16

_NC = None


def _build():
    nc = bacc.Bacc("TRN2", target_bir_lowering=False, debug=False,
                   num_devices=N_CORES)
    x_d = nc.dram_tensor("xt", [128, 12 * B_FULL], BF16,
                         kind="ExternalInput").ap()
    v_d = nc.dram_tensor("vwj", [128, VWJ_LEN], BF16,
                         kind="ExternalInput").ap()
    g_d = nc.dram_tensor("g", [128, G_LEN], BF16, kind="ExternalInput").ap()
    o_d = nc.dram_tensor("out", [O, B_FULL], F32, kind="ExternalOutput").ap()

    with tile.TileContext(nc) as tc, ExitStack() as ctx:
        const = ctx.enter_context(tc.tile_pool(name="const", bufs=1))
        big = ctx.enter_context(tc.tile_pool(name="big", bufs=1))
        psW = ctx.enter_context(tc.tile_pool(name="psW", bufs=1, space="PSUM"))
        psC = ctx.enter_context(tc.tile_pool(name="psC", bufs=1, space="PSUM"))
        psT = ctx.enter_context(tc.tile_pool(name="psT", bufs=1, space="PSUM"))
        psF = ctx.enter_context(tc.tile_pool(name="psF", bufs=1, space="PSUM"))

        # ---- PE p-state warmup on junk data (runs during DMA wait) -------
        junk = const.tile([128, 256], BF16)
        nc.vector.memset(junk[:], 0.25)
        pj = psW.tile([128, 192], F32, tag="pj")
        for _ in range(NWARM):
            nc.tensor.matmul(pj[:], junk[:, 0:128], junk[:, 0:192],
                             start=True, stop=True)

        def filler(n):
            for _ in range(n):
                nc.tensor.matmul(pj[:], junk[:, 0:128], junk[:, 0:192],
                                 start=True, stop=True)

        # CTsum per q: rows z, cols (dh, 2+s) with 64-wide rows, zero borders
        CTq = [const.tile([Z, CTS_LEN], BF16, name=f"CTq{q}")
               for q in range(3)]
        for q in range(3):
            nc.vector.memset(CTq[q][:], 0.0)

        # ---- loads: vwj, g (2 halves), x (4 chunks) -- strict FIFO -------
        vwj = big.tile([128, VWJ_LEN], BF16, tag="vwj")
        nc.sync.dma_start(vwj[:], v_d)
        gs = big.tile([128, 4, NL], BF16, tag="gs")
        nc.sync.dma_start(gs[:, 0:2, :], g_d[:, 0:2 * NL])
        nc.sync.dma_start(gs[:, 2:4, :], g_d[:, 2 * NL:4 * NL])
        XT = [big.tile([128, TPC, B_FULL], BF16, tag=f"X{j}", name=f"X{j}")
              for j in range(XCHUNKS)]
        for j in range(XCHUNKS):
            w0 = TPC * B_FULL * j
            nc.sync.dma_start(XT[j][:], x_d[:, w0:w0 + TPC * B_FULL])

        vwv = vwj[:, 0:VW_LEN].rearrange("p (s t k) -> p s t k", s=2, t=4)
        Jt = vwj[:, VW_LEN:VWJ_LEN]             # [96 used, 32] = [I;I;I]

        # ---- pd = v - w in padded (p, q, z) layout, bf16; split per t ----
        pd2 = big.tile([128, 4, KO2], BF16, tag="pd2")
        nc.vector.tensor_sub(pd2[:, 0:2], vwv[:, 0, 0:2], vwv[:, 1, 0:2])
        nc.vector.tensor_sub(pd2[:, 2:4], vwv[:, 0, 2:4], vwv[:, 1, 2:4])

        # ---- CS = sum_{t,p} pd2_tp.T @ G_t[rows 2-p .. 10-p] -------------
        ps = psC.tile([96, NW], F32, tag="psC")
        for t in range(4):
            for p in range(3):
                nc.tensor.matmul(
                    ps[:],
                    pd2[:, t, 96 * p:96 * (p + 1)],
                    gs[:, t, Wo * (2 - p):Wo * (2 - p) + NW],
                    start=(t == 0 and p == 0), stop=(t == 3 and p == 2))
        psv = ps[:].rearrange("p (h s) -> p h s", s=Wo)
        # DVE psum-fp32 -> bf16 casts corrupt data on HW; keep them on ACT.
        cast_eng = [nc.scalar.copy, nc.scalar.copy, nc.scalar.copy]
        for q in range(3):
            CTv = CTq[q][:, 0:HB * RW].rearrange("p (h s) -> p h s", s=RW)
            cast_eng[q](CTv[:, :, 2:2 + Wo], psv[32 * q:32 * (q + 1)])
        filler(2)

        # ---- W^T: 12 tiny transpose-matmuls, q-shift in the lhsT view ----
        # psT[(dh2, w), z] += CTq[q][z, flat (dh2*64 + w - q + 2)] @ I32
        pst = psT.tile([128, 4 * Z], F32, tag="pst")
        # jb-outer: consecutive 3-matmul groups per 32-col region (a start
        # resets the whole bank's accumulate-valid state, so groups must
        # not interleave within the bank).
        for jb in range(4):
            for q in range(3):
                base = 128 * jb + 2 - q
                lhsT = CTq[q][:, base:base + 128]
                nc.tensor.matmul(pst[:, 32 * jb:32 * (jb + 1)],
                                 lhsT, Jt[0:Z, :],
                                 start=(q == 0), stop=(q == 2))
        WsB = big.tile([128, 4, Z], BF16, tag="WsB")
        nc.scalar.copy(WsB[:], pst[:])

        # ---- final: out^T[o, b] += W_chunk.T @ x^T_chunk over 12 chunks --
        # Two batch halves: the first half's copy-out + DMA overlap the
        # second half's matmuls.
        BH = B_FULL // 2
        obuf = const.tile([O, B_FULL], F32)
        for h in range(2):
            pf = psF.tile([O, BH], F32, tag="psF", bufs=2)
            for t12 in range(12):
                c, jb = divmod(t12, 4)
                nc.tensor.matmul(pf[:],
                                 WsB[:, jb, O * c:O * (c + 1)],
                                 XT[t12 // TPC][:, t12 % TPC,
                                                BH * h:BH * (h + 1)],
                                 start=(t12 == 0), stop=(t12 == 11))
            nc.vector.tensor_copy(obuf[:, BH * h:BH * (h + 1)], pf[:])
            nc.sync.dma_start(o_d[:, BH * h:BH * (h + 1)],
                              obuf[:, BH * h:BH * (h + 1)])
    nc.compile()
    return nc


def _get_nc():
    global _NC
    if _NC is None:
        _NC = _build()
    return _NC


def _permute_vw(a):
    """(M, 27, 10) fp32 -> [128, 4, 288] bf16 in (p, q, z=c*10+o) layout."""
    ar = a.reshape(M, 3, 3, 3, O)            # (m, c, p, q, o)
    at = ar.transpose(0, 2, 3, 1, 4).reshape(M, 3, 3, 3 * O)
    ap = np.zeros((M, 3, 3, Z), np.float32)
    ap[..., :3 * O] = at
    return ap.reshape(4, 128, KO2).transpose(1, 0, 2).astype(BF16NP)


def _shard_inputs(inputs):
    x = np.ascontiguousarray(inputs["x"], dtype=np.float32)   # (512,3,64,64)
    G = np.ascontiguousarray(inputs["G"], dtype=np.float32)   # (512,3844)
    vp = _permute_vw(np.asarray(inputs["v"], dtype=np.float32))
    wp = _permute_vw(np.asarray(inputs["w"], dtype=np.float32))
    vw = np.stack([vp, wp], axis=1).reshape(128, VW_LEN)      # [128, 2304]
    Jp = np.zeros((128, Z), np.float32)
    Jp[:96] = np.tile(np.eye(Z, dtype=np.float32), (3, 1))
    vwj = np.ascontiguousarray(
        np.concatenate([vw, Jp.astype(BF16NP)], axis=1))      # [128, 2336]
    Gim = G.reshape(M, Ho, Wo)
    in_maps = []
    for i in range(N_CORES):
        h0 = HB * i
        xb = x[:, :, h0:h0 + HB, :].reshape(B_FULL, BAND)
        xt = np.ascontiguousarray(xb.T).reshape(12, 128, B_FULL)
        xt = np.ascontiguousarray(
            xt.transpose(1, 0, 2)).reshape(128, 12 * B_FULL).astype(BF16NP)
        gsh = np.zeros((M, IW, Wo), np.float32)
        lo, hi = h0 - 2, h0 + HB          # patch-grid rows needed
        clo, chi = max(lo, 0), min(hi, Ho)
        gsh[:, clo - lo:chi - lo, :] = Gim[:, clo:chi, :]
        gb = np.ascontiguousarray(
            gsh.reshape(4, 128, NL).transpose(1, 0, 2).reshape(
                128, G_LEN)).astype(BF16NP)
        in_maps.append({"xt": xt, "vwj": vwj, "g": gb})
    return in_maps


def _run(inputs, trace=False, **kw):
    nc = _get_nc()
    in_maps = _shard_inputs(inputs)
    res = run_bass_kernel_spmd(nc, in_maps, list(range(N_CORES)),
                               trace=trace, **kw)
    acc = np.zeros((O, B_FULL), np.float64)
    for i in range(N_CORES):
        acc += res.results[i]["out"].astype(np.float64)
    return np.ascontiguousarray(acc.T).astype(np.float32), res


def kernel(**inputs) -> np.ndarray:
    return _run(inputs)[0]
